# revision 1
# baseline (speedup 1.0000x reference)
"""CrossSparseAggrNet_v2 Trainium2 kernel.

Host (numpy, exact fp32 like the reference) computes the small image-side
aggregation network (LN -> MLP -> softmax -> aggr), top-k score masks and
the `extra` dropped-token vectors.  The 8 NeuronCores then run the dominant
compute: per caption-shard, the [544 x 2048]^T @ [544 x 10240] similarity
matmul whose contraction folds the per-(image,caption) top-k penalty in via
one-hot channels, fused with a grouped max over the 40 candidate rows
(39 aggregated tokens + CLS) per image.  The host combines with the
`extra`-token similarities and the word mask to produce sims [B_v, B_t].
"""

import numpy as np

EPS = 1e-12
BIG_NEG = 1e10
ATTN_W = 0.8
KEEPED = 39
NUM_KEEP = 19
DIM = 512
B_V = 256
B_T = 256
L_T = 64
N_CORES = 8
T_PER_CORE = B_T // N_CORES          # 32 captions per core
M_PER_CORE = T_PER_CORE * L_T        # 2048 rows (t, w)
R = 40                               # 39 aggr rows + 1 cls row per image
N_COLS = B_V * R                     # 10240
K_FEAT = DIM + T_PER_CORE            # 512 + 32 one-hot penalty channels


def _l2norm(x, axis=-1):
    n = np.sqrt(np.sum(x * x, axis=axis, keepdims=True))
    return x / np.maximum(n, EPS)


def _gelu(x):
    from scipy.special import erf
    return 0.5 * x * (1.0 + erf(x / np.sqrt(2.0).astype(np.float32)))


def _softmax(x, axis=-1):
    m = np.max(x, axis=axis, keepdims=True)
    e = np.exp(x - m)
    return e / np.sum(e, axis=axis, keepdims=True)


def _host_prep(img_embs, cap_embs, cap_lens, ln_g, ln_b, W1, b1, W2, b2, scale):
    img_embs = np.asarray(img_embs, np.float32)
    cap_embs = np.asarray(cap_embs, np.float32)
    cap_lens = np.asarray(cap_lens)
    ln_g = np.asarray(ln_g, np.float32)
    ln_b = np.asarray(ln_b, np.float32)
    W1 = np.asarray(W1, np.float32)
    b1 = np.asarray(b1, np.float32)
    W2 = np.asarray(W2, np.float32)
    b2 = np.asarray(b2, np.float32)
    scale = np.asarray(scale, np.float32)

    img_cls = img_embs[:, 0, :]                       # [B_v, C]
    spatial = img_embs[:, 1:, :]                      # [B_v, 196, C]

    # token aggregation (exact fp32, mirrors reference)
    mu = np.mean(spatial, axis=-1, keepdims=True)
    var = np.mean(np.square(spatial - mu), axis=-1, keepdims=True)
    h = (spatial - mu) / np.sqrt(var + 1e-5) * ln_g + ln_b
    h = _gelu((h.reshape(-1, DIM) @ W1 + b1).astype(np.float32)).astype(np.float32)
    w = (h @ W2 + b2).reshape(B_V, 196, KEEPED)
    w = np.swapaxes(w, 1, 2) * scale                  # [B_v, 39, 196]
    w = _softmax(w, axis=2).astype(np.float32)
    aggr = np.einsum('bkl,blc->bkc', w, spatial, optimize=True).astype(np.float32)

    aggr_norm = _l2norm(aggr)                         # [B_v, 39, C]
    cap_norm = _l2norm(cap_embs)                      # [B_t, L_t, C]
    cls_norm = _l2norm(img_cls)                       # [B_v, C]

    glo = _l2norm(np.mean(aggr, axis=1))              # [B_v, C]
    att_self = np.einsum('bc,bkc->bk', glo, aggr_norm).astype(np.float32)

    word_mask = (np.arange(L_T)[None, :] < cap_lens[:, None]).astype(np.float32)
    nw = np.sum(word_mask, axis=1)                    # [B_t]
    cap_glo = _l2norm(
        np.sum(cap_embs * word_mask[:, :, None], axis=1) / nw[:, None]
    )                                                 # [B_t, C]

    att_y = np.einsum('tc,bkc->btk', cap_glo, aggr_norm).astype(np.float32)
    score = ATTN_W * att_y + (1.0 - ATTN_W) * att_self[:, None, :]  # [B_v,B_t,39]

    # top-19 of 39 per (b, t): mask of selected entries
    thr = np.partition(score, KEEPED - NUM_KEEP, axis=-1)[..., KEEPED - NUM_KEEP]
    sel_mask = score >= thr[..., None]                # [B_v, B_t, 39] ~19 True
    # fix any tie-induced over-selection to exactly 19 (rare/never for randn)
    cnt = sel_mask.sum(-1)
    if np.any(cnt != NUM_KEEP):
        order = np.argsort(-score, axis=-1, kind='stable')
        sel_mask = np.zeros_like(sel_mask)
        np.put_along_axis(sel_mask, order[..., :NUM_KEEP], True, axis=-1)

    w_drop = _softmax(score - sel_mask * BIG_NEG, axis=-1).astype(np.float32)
    extra = np.einsum('btk,bkc->btc', w_drop, aggr, optimize=True).astype(np.float32)
    extra_n = _l2norm(extra)                          # [B_v, B_t, C]

    # image-side feature matrix for the device matmul: [512, B_v*40]
    F = np.empty((B_V, R, DIM), np.float32)
    F[:, :KEEPED] = aggr_norm
    F[:, KEEPED] = cls_norm
    imgbase = np.ascontiguousarray(F.reshape(N_COLS, DIM).T)     # [512, 10240]

    # per-core penalty rows [32, 10240] and caption features [544, 2048]
    imgpens, capfeats = [], []
    onehot = np.kron(np.eye(T_PER_CORE, dtype=np.float32),
                     np.ones((1, L_T), np.float32))   # [32, 2048]
    for c in range(N_CORES):
        tsl = slice(c * T_PER_CORE, (c + 1) * T_PER_CORE)
        P = np.zeros((T_PER_CORE, B_V, R), np.float32)
        P[:, :, :KEEPED] = np.where(
            np.transpose(sel_mask[:, tsl], (1, 0, 2)), 0.0, -BIG_NEG
        )
        imgpens.append(np.ascontiguousarray(P.reshape(T_PER_CORE, N_COLS)))
        cf = np.concatenate(
            [cap_norm[tsl].reshape(M_PER_CORE, DIM).T, onehot], axis=0
        )
        capfeats.append(np.ascontiguousarray(cf.astype(np.float32)))  # [544,2048]

    return dict(imgbase=imgbase, imgpens=imgpens, capfeats=capfeats,
                cap_norm=cap_norm, extra_n=extra_n, word_mask=word_mask, nw=nw)


def _host_smax(prep):
    """Fallback: [16384 (t,w), 256 b] masked group-max on host."""
    out = np.empty((B_T, L_T, B_V), np.float32)
    imgbase = prep['imgbase']                          # [512, 10240]
    for c in range(N_CORES):
        cf = prep['capfeats'][c]                       # [544, 2048]
        S = cf[:DIM].T @ imgbase                       # [2048, 10240]
        S += cf[DIM:].T @ prep['imgpens'][c]
        S = S.reshape(M_PER_CORE, B_V, R).max(axis=-1)  # [2048, 256]
        out[c * T_PER_CORE:(c + 1) * T_PER_CORE] = S.reshape(T_PER_CORE, L_T, B_V)
    return out


def _device_smax(prep):
    from contextlib import ExitStack
    import concourse.bass as bass
    import concourse.tile as tile
    from concourse import bacc, mybir
    from concourse.bass_utils import run_bass_kernel_spmd

    nc = bacc.Bacc("TRN2", target_bir_lowering=False, debug=False,
                   enable_asserts=False, num_devices=N_CORES)
    f32 = mybir.dt.float32
    imgbase = nc.dram_tensor("imgbase", [DIM, N_COLS], f32, kind="ExternalInput").ap()
    imgpen = nc.dram_tensor("imgpen", [T_PER_CORE, N_COLS], f32, kind="ExternalInput").ap()
    capfeat = nc.dram_tensor("capfeat", [K_FEAT, M_PER_CORE], f32, kind="ExternalInput").ap()
    smax_out = nc.dram_tensor("smax", [M_PER_CORE, B_V], f32, kind="ExternalOutput").ap()

    KT = [(0, 128), (128, 128), (256, 128), (384, 128), (512, T_PER_CORE)]
    NB = 12                      # image groups (of 40 cols) per N-chunk
    chunks = []
    b0 = 0
    while b0 < B_V:
        nb = min(NB, B_V - b0)
        chunks.append((b0, nb))
        b0 += nb

    with tile.TileContext(nc) as tc, ExitStack() as ctx:
        cfp = ctx.enter_context(tc.tile_pool(name="cf", bufs=1))
        imp = ctx.enter_context(tc.tile_pool(name="im", bufs=3))
        psp = ctx.enter_context(tc.tile_pool(name="ps", bufs=8, space="PSUM"))
        smp = ctx.enter_context(tc.tile_pool(name="sm", bufs=1))

        cft = []
        for i, (k0, kn) in enumerate(KT):
            t = cfp.tile([128, M_PER_CORE], f32, name=f"cf{i}", tag=f"cf{i}")
            src = capfeat[k0:k0 + kn, :]
            nc.sync.dma_start(t[:kn, :], src)
            cft.append(t)

        smax_tiles = [smp.tile([128, B_V], f32, name=f"sm{m}", tag=f"sm{m}")
                      for m in range(16)]

        for (b0, nb) in chunks:
            wdt = nb * R
            c0 = b0 * R
            imt = []
            for i, (k0, kn) in enumerate(KT):
                t = imp.tile([128, NB * R], f32, name=f"im{i}", tag=f"im{i}")
                src = imgpen[:, c0:c0 + wdt] if i == 4 else \
                    imgbase[k0:k0 + kn, c0:c0 + wdt]
                nc.sync.dma_start(t[:kn, :wdt], src)
                imt.append(t)
            for m in range(16):
                ps = psp.tile([128, NB * R], f32, name="ps", tag="ps")
                for i, (k0, kn) in enumerate(KT):
                    nc.tensor.matmul(
                        ps[:, :wdt],
                        cft[i][:kn, m * 128:(m + 1) * 128],
                        imt[i][:kn, :wdt],
                        start=(i == 0), stop=(i == len(KT) - 1),
                    )
                view = ps[:, :wdt].rearrange("p (b r) -> p b r", r=R)
                nc.vector.reduce_max(smax_tiles[m][:, b0:b0 + nb], view,
                                     axis=mybir.AxisListType.X)

        for m in range(16):
            nc.sync.dma_start(smax_out[m * 128:(m + 1) * 128, :], smax_tiles[m][:])

    in_maps = [
        {"imgbase": prep['imgbase'], "imgpen": prep['imgpens'][c],
         "capfeat": prep['capfeats'][c]}
        for c in range(N_CORES)
    ]
    res = run_bass_kernel_spmd(nc, in_maps, core_ids=list(range(N_CORES)))
    out = np.empty((B_T, L_T, B_V), np.float32)
    for c in range(N_CORES):
        out[c * T_PER_CORE:(c + 1) * T_PER_CORE] = \
            np.asarray(res.results[c]["smax"]).reshape(T_PER_CORE, L_T, B_V)
    return out


def kernel(**inputs):
    prep = _host_prep(**inputs)
    try:
        import signal

        def _timeout(signum, frame):
            raise TimeoutError("device path exceeded time budget")

        old_h = None
        try:
            old_h = signal.signal(signal.SIGALRM, _timeout)
            signal.alarm(240)
        except (ValueError, OSError):
            old_h = None
        try:
            smax = _device_smax(prep)                  # [B_t, L_t, B_v]
        finally:
            if old_h is not None:
                signal.alarm(0)
                signal.signal(signal.SIGALRM, old_h)
    except Exception as e:  # fall back to host so the answer is still right
        import traceback
        traceback.print_exc()
        print(f"[kernel] device path failed ({e!r}); using host fallback")
        smax = _host_smax(prep)

    # esim[t, w, b] = cap_norm[t, w] . extra_n[b, t]
    esim = np.einsum('twc,btc->twb', prep['cap_norm'], prep['extra_n'],
                     optimize=True).astype(np.float32)
    sim_max = np.maximum(smax, esim)                   # [B_t, L_t, B_v]
    sim_max *= prep['word_mask'][:, :, None]
    sims = np.sum(sim_max, axis=1) / prep['nw'][:, None]   # [B_t, B_v]
    return np.ascontiguousarray(sims.T.astype(np.float32))  # [B_v, B_t]



# revision 2
# speedup vs baseline: 3.5890x; 3.5890x over previous
"""CrossSparseAggrNet_v2 Trainium2 kernel (caption-sharded, 8 cores).

Host (numpy f32, exact selection semantics): image-side aggregation net
(LN -> gelu MLP -> softmax -> weighted sum), all l2 norms, attention
scores, per-(image,caption) top-19-of-39 selection, dropped-token
softmax weights, and the ||extra|| norms (via per-image Gram matrices).

Device (one SPMD dispatch on 8 NeuronCores): each core owns 32 captions.
The image feature matrix F = [aggr_norm | cls_norm] is sharded by image
across cores (2.6MB/core) and AllGathered on-chip instead of being
replicated through the slow host->device link.  One fused f32 matmul
computes S = cap_norm^T @ F with the top-k penalty (-30 on unselected
rows) folded in via 32 one-hot K-channels; vector engines then take the
grouped max over the 40 candidate rows (selected tokens + CLS), the
grouped weighted sum q = sum_r v*S (which equals cap.extra since
cap.extra = sum_k w_drop*||aggr||*S_norm), and the final word-masked
mean is a small mask matmul.  Output per core: [32, 256] f32.

The Bass program is built and compiled at import time (with a jax
persistent compilation cache) so kernel() itself only pays host prep +
transfer + execute.
"""

import os
import numpy as np

os.environ.setdefault("OMP_NUM_THREADS", "1")

import jax

jax.config.update("jax_compilation_cache_dir", "/tmp/jax_bass_cache")
jax.config.update("jax_persistent_cache_min_entry_size_bytes", -1)
jax.config.update("jax_persistent_cache_min_compile_time_secs", 0.0)

from contextlib import ExitStack

import concourse.bass as bass
import concourse.tile as tile
from concourse import bacc, mybir
from concourse.bass_utils import run_bass_kernel_spmd

EPS = 1e-12
BIG_NEG = 1e10
PEN = -30.0          # device-side top-k penalty (small => exact q recovery)
ATTN_W = 0.8
KEEPED = 39
NUM_KEEP = 19
DIM = 512
B_V = 256
B_T = 256
L_T = 64
L_SP = 196
HIDDEN = 102
R = 40               # 39 aggr rows + 1 cls row per image
NCOL = B_V * R       # 10240
N_CORES = 8
TPC = B_T // N_CORES          # 32 captions per core
MROWS = TPC * L_T             # 2048 rows, (w, t) ordering: row = w*TPC + t
FSH_COLS = (B_V // N_CORES) * R  # 1280 image-feature columns per shard

_f16 = mybir.dt.float16
_f32 = mybir.dt.float32

# image chunks for the device loop: 21 x 12 images + 1 x 4 images
_CHUNKS = []
_b0 = 0
while _b0 < B_V:
    _nb = min(12, B_V - _b0)
    _CHUNKS.append((_b0, _nb))
    _b0 += _nb


def _build_program():
    nc = bacc.Bacc("TRN2", target_bir_lowering=False, debug=False,
                   enable_asserts=False, num_devices=N_CORES)
    fsh = nc.dram_tensor("fsh", [DIM, FSH_COLS], _f32, kind="ExternalInput").ap()
    capT = nc.dram_tensor("capT", [DIM, MROWS], _f32, kind="ExternalInput").ap()
    oneh = nc.dram_tensor("oneh", [TPC, MROWS], _f16, kind="ExternalInput").ap()
    pen = nc.dram_tensor("pen", [TPC, NCOL], _f16, kind="ExternalInput").ap()
    vw = nc.dram_tensor("vw", [TPC, NCOL], _f16, kind="ExternalInput").ap()
    cons = nc.dram_tensor("cons", [2 * TPC, B_V], _f32, kind="ExternalInput").ap()
    mk = nc.dram_tensor("mk", [MROWS, TPC], _f32, kind="ExternalInput").ap()
    sims = nc.dram_tensor("sims", [TPC, B_V], _f32, kind="ExternalOutput").ap()

    mult = mybir.AluOpType.mult
    sub = mybir.AluOpType.subtract
    vmax = mybir.AluOpType.max
    X = mybir.AxisListType.X

    with tile.TileContext(nc) as tc, ExitStack() as ctx:
        dram = ctx.enter_context(tc.tile_pool(name="dram", bufs=1, space="DRAM"))
        big = ctx.enter_context(tc.tile_pool(name="big", bufs=1))
        fcp = ctx.enter_context(tc.tile_pool(name="fc", bufs=8))
        psp = ctx.enter_context(tc.tile_pool(name="ps", bufs=4, space="PSUM"))
        acc = ctx.enter_context(tc.tile_pool(name="acc", bufs=1, space="PSUM"))
        scrp = ctx.enter_context(tc.tile_pool(name="scr", bufs=3))
        esp = ctx.enter_context(tc.tile_pool(name="es", bufs=3))

        # --- AllGather the image-feature shards, reshuffle to [512, NCOL] ---
        gin = dram.tile([DIM, FSH_COLS], _f32, name="gin", tag="gin")
        gout = dram.tile([N_CORES * DIM, FSH_COLS], _f32, name="gout", tag="gout")
        fgat = dram.tile([DIM, NCOL], _f32, name="fgat", tag="fgat")
        nc.gpsimd.dma_start(gin[:], fsh)
        nc.gpsimd.collective_compute(
            "AllGather", mybir.AluOpType.bypass,
            replica_groups=[list(range(N_CORES))],
            ins=[gin.opt()], outs=[gout.opt()],
        )
        for g in range(N_CORES):
            for p in range(4):
                nc.sync.dma_start(
                    fgat[128 * p:128 * (p + 1), FSH_COLS * g:FSH_COLS * (g + 1)],
                    gout[DIM * g + 128 * p:DIM * g + 128 * (p + 1), :])

        # --- load caption-side operands --------------------------------
        ct = []
        for p in range(4):
            t = big.tile([128, MROWS], _f32, name=f"ct{p}", tag=f"ct{p}")
            nc.sync.dma_start(t[:], capT[128 * p:128 * (p + 1), :])
            ct.append(t)
        oh = big.tile([TPC, MROWS], _f16, name="oh", tag="oh")
        nc.sync.dma_start(oh[:], oneh)
        pen_sb = big.tile([TPC, NCOL], _f16, name="pen", tag="pen")
        nc.sync.dma_start(pen_sb[:], pen)
        v_bc = big.tile([128, NCOL], _f16, name="vbc", tag="vbc")
        inv_bc = big.tile([128, B_V], _f32, name="inv", tag="inv")
        cor_bc = big.tile([128, B_V], _f32, name="cor", tag="cor")
        for i in range(4):
            nc.sync.dma_start(v_bc[TPC * i:TPC * (i + 1), :], vw)
            nc.sync.dma_start(inv_bc[TPC * i:TPC * (i + 1), :], cons[0:TPC, :])
            nc.sync.dma_start(cor_bc[TPC * i:TPC * (i + 1), :], cons[TPC:2 * TPC, :])
        mk_sb = big.tile([128, 16 * TPC], _f32, name="mk", tag="mk")
        for m in range(16):
            nc.sync.dma_start(mk_sb[:, TPC * m:TPC * (m + 1)],
                              mk[128 * m:128 * (m + 1), :])

        qf = [big.tile([128, B_V], _f32, name=f"qf{m}", tag=f"qf{m}")
              for m in range(16)]
        sf = [big.tile([128, B_V], _f32, name=f"sf{m}", tag=f"sf{m}")
              for m in range(16)]
        sims_ps = acc.tile([TPC, B_V], _f32, name="sacc", tag="sacc")

        # --- main loop: 22 image chunks x 16 row-tiles -----------------
        for (b0, nb) in _CHUNKS:
            w = nb * R
            c0 = b0 * R
            fc = []
            for p in range(4):
                t = fcp.tile([128, 12 * R], _f32, name=f"fc{p}", tag=f"fc{p}")
                nc.sync.dma_start(t[:, :w], fgat[128 * p:128 * (p + 1), c0:c0 + w])
                fc.append(t)
            for mt in range(16):
                ms = slice(128 * mt, 128 * (mt + 1))
                ps = psp.tile([128, 12 * R], _f32, name="ps", tag="ps")
                for p in range(4):
                    nc.tensor.matmul(ps[:, :w], ct[p][:, ms], fc[p][:, :w],
                                     start=(p == 0), stop=False)
                nc.tensor.matmul(ps[:, :w], oh[:, ms], pen_sb[:, c0:c0 + w],
                                 start=False, stop=True)
                scr = scrp.tile([128, 12 * R], _f32, name="scr", tag="scr")
                nc.vector.tensor_tensor(scr[:, :w], ps[:, :w],
                                        v_bc[:, c0:c0 + w], op=mult)
                nc.vector.reduce_sum(
                    qf[mt][:, b0:b0 + nb],
                    scr[:, :w].rearrange("p (b r) -> p b r", r=R), axis=X)
                nc.vector.reduce_max(
                    sf[mt][:, b0:b0 + nb],
                    ps[:, :w].rearrange("p (b r) -> p b r", r=R), axis=X)

        for mt in range(16):
            e1 = esp.tile([128, B_V], _f32, name="e1", tag="e1")
            e2 = esp.tile([128, B_V], _f32, name="e2", tag="e2")
            e3 = esp.tile([128, B_V], _f32, name="e3", tag="e3")
            nc.vector.tensor_tensor(e1[:], qf[mt][:], inv_bc[:], op=mult)
            nc.vector.tensor_tensor(e2[:], e1[:], cor_bc[:], op=sub)
            nc.vector.tensor_tensor(e3[:], e2[:], sf[mt][:], op=vmax)
            nc.tensor.matmul(sims_ps[:], mk_sb[:, TPC * mt:TPC * (mt + 1)],
                             e3[:], start=(mt == 0), stop=(mt == 15))
        sims_sb = esp.tile([TPC, B_V], _f32, name="so", tag="so")
        nc.scalar.copy(sims_sb[:], sims_ps[:])
        nc.sync.dma_start(sims, sims_sb[:])
    nc.finalize()
    return nc


_NC = None
_DEVICE_OK = False


def _zero_in_maps():
    z = {
        "fsh": np.zeros((DIM, FSH_COLS), np.float32),
        "capT": np.zeros((DIM, MROWS), np.float32),
        "oneh": np.zeros((TPC, MROWS), np.float16),
        "pen": np.zeros((TPC, NCOL), np.float16),
        "vw": np.zeros((TPC, NCOL), np.float16),
        "cons": np.zeros((2 * TPC, B_V), np.float32),
        "mk": np.zeros((MROWS, TPC), np.float32),
    }
    return [dict(z) for _ in range(N_CORES)]


def _init_device():
    global _NC, _DEVICE_OK
    try:
        _NC = _build_program()
        run_bass_kernel_spmd(_NC, _zero_in_maps(), core_ids=list(range(N_CORES)))
        _DEVICE_OK = True
    except Exception as e:  # pragma: no cover - defensive
        import traceback
        traceback.print_exc()
        print(f"[kernel] device init failed ({e!r}); will use host fallback")
        _DEVICE_OK = False


_init_device()


def _l2n(x, axis=-1):
    n = np.sqrt(np.sum(x * x, axis=axis, keepdims=True))
    return x / np.maximum(n, EPS)


def _host_prep(img_embs, cap_embs, cap_lens, ln_g, ln_b, W1, b1, W2, b2, scale):
    """All f32 selection math."""
    img_embs = np.ascontiguousarray(np.asarray(img_embs, np.float32))
    cap_embs = np.ascontiguousarray(np.asarray(cap_embs, np.float32))
    cap_lens = np.asarray(cap_lens)
    ln_g = np.asarray(ln_g, np.float32)
    ln_b = np.asarray(ln_b, np.float32)
    W1 = np.asarray(W1, np.float32)
    b1 = np.asarray(b1, np.float32)
    W2 = np.asarray(W2, np.float32)
    b2 = np.asarray(b2, np.float32)
    scale = np.asarray(scale, np.float32)

    img_cls = img_embs[:, 0, :]                       # [B_v, C]
    spatial = img_embs[:, 1:, :]                      # [B_v, 196, C]

    # ---- aggregation net (layernorm -> gelu MLP -> softmax -> aggr) ----
    x = spatial.reshape(-1, DIM)                      # [50176, 512]
    mu = x.mean(axis=1, keepdims=True)
    xc = x - mu
    var = np.einsum('ij,ij->i', xc, xc, optimize=True)[:, None] / DIM
    h = xc / np.sqrt(var + 1e-5)
    h = h * ln_g + ln_b
    a1 = h @ W1
    a1 += b1
    from scipy.special import erf
    a1 = (0.5 * a1 * (1.0 + erf(a1 * np.float32(0.7071067811865476)))).astype(np.float32)
    w = a1 @ W2
    w += b2                                           # [50176, 39]
    w = w.reshape(B_V, L_SP, KEEPED) * scale          # softmax over L_SP
    w -= w.max(axis=1, keepdims=True)
    np.exp(w, out=w)
    w /= w.sum(axis=1, keepdims=True)
    aggr = np.matmul(w.transpose(0, 2, 1), spatial)   # [B_v, 39, C]

    norms = np.sqrt(np.einsum('bkc,bkc->bk', aggr, aggr, optimize=True))
    norms_c = np.maximum(norms, EPS)
    aggr_n = aggr / norms_c[:, :, None]
    cls_n = _l2n(img_cls)
    glo = _l2n(aggr.mean(axis=1))
    att_self = np.einsum('bc,bkc->bk', glo, aggr_n, optimize=True)

    # ---- caption side --------------------------------------------------
    wm = (np.arange(L_T)[None, :] < cap_lens[:, None]).astype(np.float32)
    nw = wm.sum(axis=1)                               # [B_t]
    capsum = np.matmul(wm[:, None, :], cap_embs)[:, 0]  # [B_t, C]
    cap_glo = _l2n(capsum / nw[:, None])
    cn = np.sqrt(np.einsum('twc,twc->tw', cap_embs, cap_embs, optimize=True))
    cap_norm = cap_embs / np.maximum(cn, EPS)[:, :, None]

    # ---- scores, top-k, dropped-token weights -------------------------
    att_y = cap_glo @ aggr_n.reshape(-1, DIM).T       # [B_t, 9984]
    score = ATTN_W * att_y.reshape(B_T, B_V, KEEPED) \
        + (1.0 - ATTN_W) * att_self[None]             # [t, b, 39]
    kth = KEEPED - NUM_KEEP                           # 20
    thr = np.partition(score, kth, axis=-1)[..., kth]
    sel = score >= thr[..., None]                     # [t, b, 39]
    cnt = sel.sum(-1)
    if np.any(cnt != NUM_KEEP):                       # tie fixup (rare)
        order = np.argsort(-score, axis=-1, kind='stable')
        sel = np.zeros_like(sel)
        np.put_along_axis(sel, order[..., :NUM_KEEP], True, axis=-1)
    with np.errstate(under='ignore'):
        wd = score - sel * np.float32(BIG_NEG)
        wd -= wd.max(axis=-1, keepdims=True)
        np.exp(wd, out=wd)
        wd /= wd.sum(axis=-1, keepdims=True)          # w_drop [t, b, 39]
    v3 = wd * norms_c[None]                           # [t, b, 39]
    v16 = v3.astype(np.float16)                       # exactly what device sums

    # ---- ||extra|| via per-image Gram matrices ------------------------
    G = np.matmul(aggr, aggr.transpose(0, 2, 1))      # [b, 39, 39]
    wd_b = np.ascontiguousarray(wd.transpose(1, 0, 2))  # [b, t, 39]
    H = np.matmul(wd_b, G)                            # [b, t, 39]
    e2 = np.einsum('btk,btk->bt', H, wd_b, optimize=True)
    inv_en = (1.0 / np.maximum(np.sqrt(np.maximum(e2, 0.0)), EPS)).T  # [t, b]
    corr2 = (np.float32(PEN) * v16.astype(np.float32).sum(-1)) * inv_en

    return dict(aggr_n=aggr_n, cls_n=cls_n, cap_norm=cap_norm, wm=wm, nw=nw,
                sel=sel, wd=wd, v3=v3, v16=v16, inv_en=inv_en, corr2=corr2,
                norms_c=norms_c)


_ONEHOT = np.tile(np.eye(TPC, dtype=np.float16), (1, L_T))  # [32, 2048]


def _pack_inputs(prep):
    aggr_n, cls_n = prep['aggr_n'], prep['cls_n']
    F = np.empty((B_V, R, DIM), np.float32)
    F[:, :KEEPED] = aggr_n
    F[:, KEEPED] = cls_n

    pen_all = np.full((B_T, B_V, R), np.float16(PEN), np.float16)
    pen_all[:, :, :KEEPED][prep['sel']] = np.float16(0.0)
    pen_all[:, :, KEEPED] = np.float16(0.0)
    v_all = np.zeros((B_T, B_V, R), np.float16)
    v_all[:, :, :KEEPED] = prep['v16']

    base = (prep['wm'] / prep['nw'][:, None]).astype(np.float32)  # [B_t, L_T]

    idx = np.arange(TPC)
    in_maps = []
    for c in range(N_CORES):
        tsl = slice(TPC * c, TPC * (c + 1))
        bsl = slice((B_V // N_CORES) * c, (B_V // N_CORES) * (c + 1))
        fsh = np.ascontiguousarray(
            F[bsl].reshape(FSH_COLS, DIM).T)                      # [512, 1280]
        capTm = np.ascontiguousarray(
            prep['cap_norm'][tsl].transpose(2, 1, 0).reshape(DIM, MROWS))
        mkm = np.zeros((L_T, TPC, TPC), np.float32)
        mkm[:, idx, idx] = base[tsl].T                            # [w, t, t]
        cons = np.concatenate([prep['inv_en'][tsl], prep['corr2'][tsl]],
                              axis=0).astype(np.float32)          # [64, 256]
        in_maps.append({
            "fsh": fsh,
            "capT": capTm,
            "oneh": _ONEHOT,
            "pen": np.ascontiguousarray(pen_all[tsl].reshape(TPC, NCOL)),
            "vw": np.ascontiguousarray(v_all[tsl].reshape(TPC, NCOL)),
            "cons": cons,
            "mk": mkm.reshape(MROWS, TPC),
        })
    return in_maps


def _host_sims(prep):
    """Pure-host fallback: exact f32 computation of sims [B_t, B_v]."""
    F = np.empty((B_V, R, DIM), np.float32)
    F[:, :KEEPED] = prep['aggr_n']
    F[:, KEEPED] = prep['cls_n']
    Fm = F.reshape(NCOL, DIM).T                       # [512, 10240]
    capf = prep['cap_norm'].reshape(B_T * L_T, DIM)   # [(t,w), 512]
    pen_all = np.full((B_T, B_V, R), -1e4, np.float32)
    pen_all[:, :, :KEEPED][prep['sel']] = 0.0
    pen_all[:, :, KEEPED] = 0.0
    v_all = np.zeros((B_T, B_V, R), np.float32)
    v_all[:, :, :KEEPED] = prep['v3']
    sims = np.empty((B_T, B_V), np.float32)
    blk = 32
    for t0 in range(0, B_T, blk):
        S = capf[t0 * L_T:(t0 + blk) * L_T] @ Fm      # [blk*64, 10240]
        S = S.reshape(blk, L_T, B_V, R)
        q = np.einsum('twbr,tbr->twb', S, v_all[t0:t0 + blk], optimize=True)
        esim = q * prep['inv_en'][t0:t0 + blk, None, :]
        Sp = S + pen_all[t0:t0 + blk, :, None, :].transpose(0, 2, 1, 3)
        smax = Sp.max(axis=-1)                        # [blk, 64, B_v]
        simw = np.maximum(smax, esim)
        simw *= prep['wm'][t0:t0 + blk, :, None]
        sims[t0:t0 + blk] = simw.sum(axis=1) / prep['nw'][t0:t0 + blk, None]
    return sims


def kernel(**inputs):
    prep = _host_prep(**inputs)
    sims = None
    if _DEVICE_OK:
        try:
            in_maps = _pack_inputs(prep)
            res = run_bass_kernel_spmd(_NC, in_maps, core_ids=list(range(N_CORES)))
            sims = np.concatenate(
                [np.asarray(res.results[c]["sims"]) for c in range(N_CORES)],
                axis=0)                                # [B_t, B_v]
        except Exception as e:
            import traceback
            traceback.print_exc()
            print(f"[kernel] device path failed ({e!r}); using host fallback")
            sims = None
    if sims is None:
        sims = _host_sims(prep)
    return np.ascontiguousarray(sims.T.astype(np.float32))  # [B_v, B_t]


# revision 3
# speedup vs baseline: 4.6679x; 1.3006x over previous
"""CrossSparseAggrNet_v2 Trainium2 kernel (caption-sharded, 8 cores).

Host (numpy f32, exact selection semantics): image-side aggregation net
(LN -> gelu MLP -> softmax -> weighted sum), all l2 norms, attention
scores, per-(image,caption) top-19-of-39 selection, dropped-token
softmax weights, and the ||extra|| norms (via per-image Gram matrices).

Device (one SPMD dispatch on 8 NeuronCores): each core owns 32 captions.
The image feature matrix F = [aggr_norm | cls_norm] is sharded by image
across cores (2.6MB/core) and AllGathered on-chip instead of being
replicated through the slow host->device link.  One fused f32 matmul
computes S = cap_norm^T @ F with the top-k penalty (-30 on unselected
rows) folded in via 32 one-hot K-channels; vector engines then take the
grouped max over the 40 candidate rows (selected tokens + CLS), the
grouped weighted sum q = sum_r v*S (which equals cap.extra since
cap.extra = sum_k w_drop*||aggr||*S_norm), and the final word-masked
mean is a small mask matmul.  Output per core: [32, 256] f32.

The Bass program is built and compiled at import time (with a jax
persistent compilation cache).  kernel() interleaves host prep with
async device_put of each operand as soon as it is ready, so the
host->device copies ride under the numpy compute.
"""

import os
import numpy as np

os.environ.setdefault("OMP_NUM_THREADS", "1")

import jax

jax.config.update("jax_compilation_cache_dir", "/tmp/jax_bass_cache")
jax.config.update("jax_persistent_cache_min_entry_size_bytes", -1)
jax.config.update("jax_persistent_cache_min_compile_time_secs", 0.0)

from contextlib import ExitStack

import concourse.bass as bass
import concourse.tile as tile
from concourse import bacc, bass2jax, mybir
from jax.experimental.shard_map import shard_map
from jax.sharding import Mesh, NamedSharding, PartitionSpec

EPS = 1e-12
BIG_NEG = 1e10
PEN = -30.0          # device-side top-k penalty (small => exact q recovery)
ATTN_W = 0.8
KEEPED = 39
NUM_KEEP = 19
DIM = 512
B_V = 256
B_T = 256
L_T = 64
L_SP = 196
HIDDEN = 102
R = 40               # 39 aggr rows + 1 cls row per image
NCOL = B_V * R       # 10240
N_CORES = 8
TPC = B_T // N_CORES          # 32 captions per core
MROWS = TPC * L_T             # 2048 rows, (w, t) ordering: row = w*TPC + t
BPC = B_V // N_CORES          # 32 images per shard
FSH_COLS = BPC * R            # 1280 image-feature columns per shard

_f16 = mybir.dt.float16
_f32 = mybir.dt.float32

# image chunks for the device loop: 21 x 12 images + 1 x 4 images
_CHUNKS = []
_b0 = 0
while _b0 < B_V:
    _nb = min(12, B_V - _b0)
    _CHUNKS.append((_b0, _nb))
    _b0 += _nb


def _build_program():
    nc = bacc.Bacc("TRN2", target_bir_lowering=False, debug=False,
                   enable_asserts=False, num_devices=N_CORES)
    fsh = nc.dram_tensor("fsh", [DIM, FSH_COLS], _f32, kind="ExternalInput").ap()
    capT = nc.dram_tensor("capT", [DIM, MROWS], _f32, kind="ExternalInput").ap()
    oneh = nc.dram_tensor("oneh", [TPC, MROWS], _f16, kind="ExternalInput").ap()
    pen = nc.dram_tensor("pen", [TPC, NCOL], _f16, kind="ExternalInput").ap()
    vw = nc.dram_tensor("vw", [TPC, NCOL], _f16, kind="ExternalInput").ap()
    cons = nc.dram_tensor("cons", [2 * TPC, B_V], _f32, kind="ExternalInput").ap()
    mk = nc.dram_tensor("mk", [MROWS, TPC], _f32, kind="ExternalInput").ap()
    sims = nc.dram_tensor("sims", [TPC, B_V], _f32, kind="ExternalOutput").ap()

    mult = mybir.AluOpType.mult
    sub = mybir.AluOpType.subtract
    vmax = mybir.AluOpType.max
    X = mybir.AxisListType.X

    with tile.TileContext(nc) as tc, ExitStack() as ctx:
        dram = ctx.enter_context(tc.tile_pool(name="dram", bufs=1, space="DRAM"))
        big = ctx.enter_context(tc.tile_pool(name="big", bufs=1))
        fcp = ctx.enter_context(tc.tile_pool(name="fc", bufs=8))
        psp = ctx.enter_context(tc.tile_pool(name="ps", bufs=4, space="PSUM"))
        acc = ctx.enter_context(tc.tile_pool(name="acc", bufs=1, space="PSUM"))
        scrp = ctx.enter_context(tc.tile_pool(name="scr", bufs=3))
        esp = ctx.enter_context(tc.tile_pool(name="es", bufs=3))

        # --- AllGather the image-feature shards, reshuffle to [512, NCOL] ---
        gin = dram.tile([DIM, FSH_COLS], _f32, name="gin", tag="gin")
        gout = dram.tile([N_CORES * DIM, FSH_COLS], _f32, name="gout", tag="gout")
        fgat = dram.tile([DIM, NCOL], _f32, name="fgat", tag="fgat")
        nc.gpsimd.dma_start(gin[:], fsh)
        nc.gpsimd.collective_compute(
            "AllGather", mybir.AluOpType.bypass,
            replica_groups=[list(range(N_CORES))],
            ins=[gin.opt()], outs=[gout.opt()],
        )
        for g in range(N_CORES):
            for p in range(4):
                nc.sync.dma_start(
                    fgat[128 * p:128 * (p + 1), FSH_COLS * g:FSH_COLS * (g + 1)],
                    gout[DIM * g + 128 * p:DIM * g + 128 * (p + 1), :])

        # --- load caption-side operands --------------------------------
        ct = []
        for p in range(4):
            t = big.tile([128, MROWS], _f32, name=f"ct{p}", tag=f"ct{p}")
            nc.sync.dma_start(t[:], capT[128 * p:128 * (p + 1), :])
            ct.append(t)
        oh = big.tile([TPC, MROWS], _f16, name="oh", tag="oh")
        nc.sync.dma_start(oh[:], oneh)
        pen_sb = big.tile([TPC, NCOL], _f16, name="pen", tag="pen")
        nc.sync.dma_start(pen_sb[:], pen)
        v_bc = big.tile([128, NCOL], _f16, name="vbc", tag="vbc")
        inv_bc = big.tile([128, B_V], _f32, name="inv", tag="inv")
        cor_bc = big.tile([128, B_V], _f32, name="cor", tag="cor")
        for i in range(4):
            nc.sync.dma_start(v_bc[TPC * i:TPC * (i + 1), :], vw)
            nc.sync.dma_start(inv_bc[TPC * i:TPC * (i + 1), :], cons[0:TPC, :])
            nc.sync.dma_start(cor_bc[TPC * i:TPC * (i + 1), :], cons[TPC:2 * TPC, :])
        mk_sb = big.tile([128, 16 * TPC], _f32, name="mk", tag="mk")
        for m in range(16):
            nc.sync.dma_start(mk_sb[:, TPC * m:TPC * (m + 1)],
                              mk[128 * m:128 * (m + 1), :])

        qf = [big.tile([128, B_V], _f32, name=f"qf{m}", tag=f"qf{m}")
              for m in range(16)]
        sf = [big.tile([128, B_V], _f32, name=f"sf{m}", tag=f"sf{m}")
              for m in range(16)]
        sims_ps = acc.tile([TPC, B_V], _f32, name="sacc", tag="sacc")

        # --- main loop: 22 image chunks x 16 row-tiles -----------------
        for (b0, nb) in _CHUNKS:
            w = nb * R
            c0 = b0 * R
            fc = []
            for p in range(4):
                t = fcp.tile([128, 12 * R], _f32, name=f"fc{p}", tag=f"fc{p}")
                nc.sync.dma_start(t[:, :w], fgat[128 * p:128 * (p + 1), c0:c0 + w])
                fc.append(t)
            for mt in range(16):
                ms = slice(128 * mt, 128 * (mt + 1))
                ps = psp.tile([128, 12 * R], _f32, name="ps", tag="ps")
                for p in range(4):
                    nc.tensor.matmul(ps[:, :w], ct[p][:, ms], fc[p][:, :w],
                                     start=(p == 0), stop=False)
                nc.tensor.matmul(ps[:, :w], oh[:, ms], pen_sb[:, c0:c0 + w],
                                 start=False, stop=True)
                scr = scrp.tile([128, 12 * R], _f32, name="scr", tag="scr")
                nc.vector.tensor_tensor(scr[:, :w], ps[:, :w],
                                        v_bc[:, c0:c0 + w], op=mult)
                nc.vector.reduce_sum(
                    qf[mt][:, b0:b0 + nb],
                    scr[:, :w].rearrange("p (b r) -> p b r", r=R), axis=X)
                nc.vector.reduce_max(
                    sf[mt][:, b0:b0 + nb],
                    ps[:, :w].rearrange("p (b r) -> p b r", r=R), axis=X)

        for mt in range(16):
            e1 = esp.tile([128, B_V], _f32, name="e1", tag="e1")
            e2 = esp.tile([128, B_V], _f32, name="e2", tag="e2")
            e3 = esp.tile([128, B_V], _f32, name="e3", tag="e3")
            nc.vector.tensor_tensor(e1[:], qf[mt][:], inv_bc[:], op=mult)
            nc.vector.tensor_tensor(e2[:], e1[:], cor_bc[:], op=sub)
            nc.vector.tensor_tensor(e3[:], e2[:], sf[mt][:], op=vmax)
            nc.tensor.matmul(sims_ps[:], mk_sb[:, TPC * mt:TPC * (mt + 1)],
                             e3[:], start=(mt == 0), stop=(mt == 15))
        sims_sb = esp.tile([TPC, B_V], _f32, name="so", tag="so")
        nc.scalar.copy(sims_sb[:], sims_ps[:])
        nc.sync.dma_start(sims, sims_sb[:])
    nc.finalize()
    return nc


def _make_runner(nc):
    """One reusable jitted executor mirroring run_bass_via_pjrt."""
    bass2jax.install_neuronx_cc_hook()
    partition_name = nc.partition_id_tensor.name if nc.partition_id_tensor else None
    in_names, out_names, out_avals = [], [], []
    for alloc in nc.m.functions[0].allocations:
        if not isinstance(alloc, mybir.MemoryLocationSet):
            continue
        name = alloc.memorylocations[0].name
        if alloc.kind == "ExternalInput":
            if name != partition_name:
                in_names.append(name)
        elif alloc.kind == "ExternalOutput":
            out_names.append(name)
            out_avals.append(jax.core.ShapedArray(
                tuple(alloc.tensor_shape), mybir.dt.np(alloc.dtype)))
    n_params = len(in_names)
    all_in = list(in_names) + list(out_names)
    if partition_name is not None:
        all_in.append(partition_name)
    donate = tuple(range(n_params, n_params + len(out_names)))

    def _body(*args):
        operands = list(args)
        if partition_name is not None:
            operands.append(bass2jax.partition_id_tensor())
        outs = bass2jax._bass_exec_p.bind(
            *operands,
            out_avals=tuple(out_avals),
            in_names=tuple(all_in),
            out_names=tuple(out_names),
            lowering_input_output_aliases=(),
            sim_require_finite=True,
            sim_require_nnan=True,
            nc=nc,
        )
        return tuple(outs)

    devices = jax.devices()[:N_CORES]
    mesh = Mesh(np.asarray(devices), ("core",))
    nin = n_params + len(out_names)
    jitted = jax.jit(
        shard_map(_body, mesh=mesh, in_specs=(PartitionSpec("core"),) * nin,
                  out_specs=(PartitionSpec("core"),) * len(out_names),
                  check_rep=False),
        donate_argnums=donate, keep_unused=True)
    sharding = NamedSharding(mesh, PartitionSpec("core"))
    return jitted, in_names, out_names, out_avals, sharding


_NC = None
_RUN = None
_DEVICE_OK = False
_ONEHOT = np.tile(np.eye(TPC, dtype=np.float16), (1, L_T))  # [32, 2048]
_ONEH_DEV = None

_IN_SHAPES = {
    "fsh": ((DIM, FSH_COLS), np.float32),
    "capT": ((DIM, MROWS), np.float32),
    "oneh": ((TPC, MROWS), np.float16),
    "pen": ((TPC, NCOL), np.float16),
    "vw": ((TPC, NCOL), np.float16),
    "cons": ((2 * TPC, B_V), np.float32),
    "mk": ((MROWS, TPC), np.float32),
}


def _init_device():
    global _NC, _RUN, _DEVICE_OK, _ONEH_DEV
    try:
        _NC = _build_program()
        _RUN = _make_runner(_NC)
        jitted, in_names, out_names, out_avals, sharding = _RUN
        _ONEH_DEV = jax.device_put(np.tile(_ONEHOT, (N_CORES, 1)), sharding)
        puts = {}
        for n in in_names:
            if n == "oneh":
                puts[n] = _ONEH_DEV
            else:
                shp, dt = _IN_SHAPES[n]
                puts[n] = jax.device_put(
                    np.zeros((N_CORES * shp[0],) + shp[1:], dt), sharding)
        zouts = [jax.device_put(
            np.zeros((N_CORES * a.shape[0],) + a.shape[1:], a.dtype), sharding)
            for a in out_avals]
        outs = jitted(*[puts[n] for n in in_names], *zouts)
        np.asarray(outs[0])
        _DEVICE_OK = True
    except Exception as e:  # pragma: no cover - defensive
        import traceback
        traceback.print_exc()
        print(f"[kernel] device init failed ({e!r}); will use host fallback")
        _DEVICE_OK = False


_init_device()


def _l2n(x, axis=-1):
    n = np.sqrt(np.sum(x * x, axis=axis, keepdims=True))
    return x / np.maximum(n, EPS)


def _host_prep(img_embs, cap_embs, cap_lens, ln_g, ln_b, W1, b1, W2, b2, scale,
               put=None):
    """All f32 selection math; calls put(name, concat_array) as operands
    become ready so transfers overlap the remaining compute."""
    img_embs = np.asarray(img_embs, np.float32)
    cap_embs = np.ascontiguousarray(np.asarray(cap_embs, np.float32))
    cap_lens = np.asarray(cap_lens)
    ln_g = np.asarray(ln_g, np.float32)
    ln_b = np.asarray(ln_b, np.float32)
    W1 = np.asarray(W1, np.float32)
    b1 = np.asarray(b1, np.float32)
    W2 = np.asarray(W2, np.float32)
    b2 = np.asarray(b2, np.float32)
    scale = np.asarray(scale, np.float32)
    if put is None:
        put = lambda name, arr: None

    # ---- caption side first (feeds the biggest transfer) --------------
    wm = (np.arange(L_T)[None, :] < cap_lens[:, None]).astype(np.float32)
    nw = wm.sum(axis=1)                               # [B_t]
    capsum = np.matmul(wm[:, None, :], cap_embs)[:, 0]  # [B_t, C]
    cap_glo = _l2n(capsum / nw[:, None])
    cn = np.sqrt(np.einsum('twc,twc->tw', cap_embs, cap_embs, optimize=True))
    cap_norm = cap_embs / np.maximum(cn, EPS)[:, :, None]
    # capT concat: [8*512, 2048], rows (w,t)-ordering within each core
    put("capT", np.ascontiguousarray(
        cap_norm.reshape(N_CORES, TPC, L_T, DIM)
        .transpose(0, 3, 2, 1).reshape(N_CORES * DIM, MROWS)))
    base = wm / nw[:, None]                           # [B_t, L_T]
    mkm = np.zeros((N_CORES, L_T, TPC, TPC), np.float32)
    idx = np.arange(TPC)
    mkm[:, :, idx, idx] = base.reshape(N_CORES, TPC, L_T).transpose(0, 2, 1)
    put("mk", mkm.reshape(N_CORES * MROWS, TPC))

    # ---- aggregation net (layernorm -> gelu MLP -> softmax -> aggr) ----
    img_cls = img_embs[:, 0, :]                       # [B_v, C]
    spatial = img_embs[:, 1:, :]                      # [B_v, 196, C]
    x = np.ascontiguousarray(spatial).reshape(-1, DIM)  # [50176, 512]
    mu = x.mean(axis=1, keepdims=True)
    xc = x - mu
    var = np.einsum('ij,ij->i', xc, xc, optimize=True)[:, None] / DIM
    h = xc / np.sqrt(var + 1e-5)
    if not (ln_g == 1.0).all():
        h *= ln_g
    if ln_b.any():
        h += ln_b
    a1 = h @ W1
    if b1.any():
        a1 += b1
    from scipy.special import erf
    a1 = (0.5 * a1 * (1.0 + erf(a1 * np.float32(0.7071067811865476)))).astype(np.float32)
    w = a1 @ W2
    if b2.any():
        w += b2                                       # [50176, 39]
    w = w.reshape(B_V, L_SP, KEEPED)
    sc = float(np.asarray(scale).reshape(-1)[0]) if scale.size == 1 else None
    if sc is None:
        w = w * scale
    elif sc != 1.0:
        w *= np.float32(sc)
    w -= w.max(axis=1, keepdims=True)                 # softmax over L_SP
    np.exp(w, out=w)
    w /= w.sum(axis=1, keepdims=True)
    aggr = np.matmul(w.transpose(0, 2, 1), spatial)   # [B_v, 39, C]

    norms = np.sqrt(np.einsum('bkc,bkc->bk', aggr, aggr, optimize=True))
    norms_c = np.maximum(norms, EPS)
    aggr_n = aggr / norms_c[:, :, None]
    cls_n = _l2n(img_cls)
    # fsh concat: [8*512, 1280]
    F = np.empty((B_V, R, DIM), np.float32)
    F[:, :KEEPED] = aggr_n
    F[:, KEEPED] = cls_n
    put("fsh", np.ascontiguousarray(
        F.reshape(N_CORES, BPC * R, DIM).transpose(0, 2, 1)
        .reshape(N_CORES * DIM, FSH_COLS)))

    glo = _l2n(aggr.mean(axis=1))
    att_self = np.einsum('bc,bkc->bk', glo, aggr_n, optimize=True)

    # ---- scores, top-k, dropped-token weights -------------------------
    att_y = cap_glo @ aggr_n.reshape(-1, DIM).T       # [B_t, 9984]
    score = ATTN_W * att_y.reshape(B_T, B_V, KEEPED) \
        + (1.0 - ATTN_W) * att_self[None]             # [t, b, 39]
    kth = KEEPED - NUM_KEEP                           # 20
    thr = np.partition(score, kth, axis=-1)[..., kth]
    sel = score >= thr[..., None]                     # [t, b, 39]
    cnt = sel.sum(-1)
    if np.any(cnt != NUM_KEEP):                       # tie fixup (rare)
        order = np.argsort(-score, axis=-1, kind='stable')
        sel = np.zeros_like(sel)
        np.put_along_axis(sel, order[..., :NUM_KEEP], True, axis=-1)
    with np.errstate(under='ignore'):
        wd = score - sel * np.float32(BIG_NEG)
        wd -= wd.max(axis=-1, keepdims=True)
        np.exp(wd, out=wd)
        wd /= wd.sum(axis=-1, keepdims=True)          # w_drop [t, b, 39]
    v3 = wd * norms_c[None]                           # [t, b, 39]
    v16 = v3.astype(np.float16)                       # exactly what device sums

    pen_all = np.full((B_T, B_V, R), np.float16(PEN), np.float16)
    pen_all[:, :, :KEEPED][sel] = np.float16(0.0)
    pen_all[:, :, KEEPED] = np.float16(0.0)
    put("pen", pen_all.reshape(B_T, NCOL))
    v_all = np.zeros((B_T, B_V, R), np.float16)
    v_all[:, :, :KEEPED] = v16
    put("vw", v_all.reshape(B_T, NCOL))

    # ---- ||extra|| via per-image Gram matrices ------------------------
    G = np.matmul(aggr, aggr.transpose(0, 2, 1))      # [b, 39, 39]
    wd_b = np.ascontiguousarray(wd.transpose(1, 0, 2))  # [b, t, 39]
    H = np.matmul(wd_b, G)                            # [b, t, 39]
    e2 = np.einsum('btk,btk->bt', H, wd_b, optimize=True)
    inv_en = (1.0 / np.maximum(np.sqrt(np.maximum(e2, 0.0)), EPS)).T  # [t, b]
    corr2 = (np.float32(PEN) * v16.astype(np.float32).sum(-1)) * inv_en
    cons = np.empty((N_CORES, 2 * TPC, B_V), np.float32)
    cons[:, :TPC] = inv_en.reshape(N_CORES, TPC, B_V)
    cons[:, TPC:] = corr2.reshape(N_CORES, TPC, B_V)
    put("cons", cons.reshape(N_CORES * 2 * TPC, B_V))

    return dict(aggr_n=aggr_n, cls_n=cls_n, cap_norm=cap_norm, wm=wm, nw=nw,
                sel=sel, wd=wd, v3=v3, v16=v16, inv_en=inv_en, corr2=corr2,
                norms_c=norms_c)


def _host_sims(prep):
    """Pure-host fallback: exact f32 computation of sims [B_t, B_v]."""
    F = np.empty((B_V, R, DIM), np.float32)
    F[:, :KEEPED] = prep['aggr_n']
    F[:, KEEPED] = prep['cls_n']
    Fm = F.reshape(NCOL, DIM).T                       # [512, 10240]
    capf = prep['cap_norm'].reshape(B_T * L_T, DIM)   # [(t,w), 512]
    pen_all = np.full((B_T, B_V, R), -1e4, np.float32)
    pen_all[:, :, :KEEPED][prep['sel']] = 0.0
    pen_all[:, :, KEEPED] = 0.0
    v_all = np.zeros((B_T, B_V, R), np.float32)
    v_all[:, :, :KEEPED] = prep['v3']
    sims = np.empty((B_T, B_V), np.float32)
    blk = 32
    for t0 in range(0, B_T, blk):
        S = capf[t0 * L_T:(t0 + blk) * L_T] @ Fm      # [blk*64, 10240]
        S = S.reshape(blk, L_T, B_V, R)
        q = np.einsum('twbr,tbr->twb', S, v_all[t0:t0 + blk], optimize=True)
        esim = q * prep['inv_en'][t0:t0 + blk, None, :]
        Sp = S + pen_all[t0:t0 + blk, :, None, :].transpose(0, 2, 1, 3)
        smax = Sp.max(axis=-1)                        # [blk, 64, B_v]
        simw = np.maximum(smax, esim)
        simw *= prep['wm'][t0:t0 + blk, :, None]
        sims[t0:t0 + blk] = simw.sum(axis=1) / prep['nw'][t0:t0 + blk, None]
    return sims


def kernel(**inputs):
    sims = None
    prep = None
    if _DEVICE_OK:
        jitted, in_names, out_names, out_avals, sharding = _RUN
        puts = {"oneh": _ONEH_DEV}

        def _put(name, arr):
            puts[name] = jax.device_put(arr, sharding)

        try:
            zouts = [jax.device_put(
                np.zeros((N_CORES * a.shape[0],) + a.shape[1:], a.dtype),
                sharding) for a in out_avals]
            prep = _host_prep(**inputs, put=_put)
            outs = jitted(*[puts[n] for n in in_names], *zouts)
            sims = np.asarray(outs[out_names.index("sims")])  # [8*32, 256]
        except Exception as e:
            import traceback
            traceback.print_exc()
            print(f"[kernel] device path failed ({e!r}); using host fallback")
            sims = None
    if sims is None:
        if prep is None:
            prep = _host_prep(**inputs)
        sims = _host_sims(prep)
    return np.ascontiguousarray(sims.T.astype(np.float32))  # [B_v, B_t]
